# revision 9
# baseline (speedup 1.0000x reference)
"""Trainium2 Bass kernel for the brute-force antisymmetrized ResNet.

Math (per walker b):
    feats[i,j] = concat(x1[P1[i]], x2[P2[j]]).reshape(24)    (576 = 24*24 perm pairs)
    y0 = tanh(feats @ W0 + b0)
    y1 = tanh(y0 @ W1 + b1) + y0
    y2 = tanh(y1 @ W2 + b2) + y1
    out[b] = log| sum_{i,j} s1[i] s2[j] (y2 @ Wf + bf) |

Strategy:
  - Data-parallel over the 512 walkers: 64 walkers per NeuronCore x 8 cores.
  - First layer is factored: y0pre[b,i,j] = u1[b,i] + u2[b,j] where
    u1[b,i] = x1feats(perm i) @ W0[:12] + b0 and u2[b,j] = x2feats(perm j) @
    W0[12:24].  The device computes u1/u2 with two tiny matmuls (24 columns
    per walker each) and broadcast-adds them into the 576 rows on VectorE.
  - Permutations are sign-sorted into quadrants so each walker's 576 rows are
    ordered [(+,+), (-,-), (+,-), (-,+)] x 12 x 12: the first 288 rows have
    pair-sign +1, the last 288 have -1, while u1/u2 accesses stay regular
    strided APs (12-blocks).
  - Activations live in [channel, row] layout: weights are the matmul
    stationary operand, activation rows stream as the moving operand.  Plain
    fp32 matmuls (fp32r/tf32 rounding is amplified catastrophically by the
    antisymmetrization: |anti| reaches 2.6e-4 against O(1) terms).
    tanh on ScalarE from PSUM, residual-1 on VectorE (in place), residual-2 on
    GpSimd into a per-group ring, per-walker sign-segment sums on VectorE.
  - Device returns v[c, walker, sign] partial sums; host applies Wf in fp64
    and log|.|.
"""

import itertools

import numpy as np

N1 = 4
N2 = 4
D = 3
BATCH = 512
NDENSE = 256
NCORES = 8
NPERM = 24              # 4!
NPAIR = NPERM * NPERM   # 576
HALFPAIR = NPAIR // 2   # 288 rows of each sign per walker
QUAD = NPERM // 2 * (NPERM // 2)            # 144 rows per quadrant
WALKERS_PER_CORE = BATCH // NCORES          # 64
ROWS_PER_CORE = WALKERS_PER_CORE * NPAIR    # 36864
TILE = 512                                  # matmul moving-dim tile
GROUP_WALKERS = 8                           # walkers per ring group
GROUP_ROWS = GROUP_WALKERS * NPAIR          # 4608 = 9 * TILE
TILES_PER_GROUP = GROUP_ROWS // TILE        # 9
NGROUPS = ROWS_PER_CORE // GROUP_ROWS       # 8
UCOLS = WALKERS_PER_CORE * NPERM            # 1536 u-columns per core
K1 = N1 * D + 1                             # 13: x1 features + ones row (b0)
K2 = N2 * D                                 # 12


def _perms_and_signs(n):
    P = np.array(list(itertools.permutations(range(n))), dtype=np.int32)
    triu = np.triu(np.ones((n, n), dtype=np.int64), 1)
    inv = np.sum((P[:, :, None] > P[:, None, :]) * triu, axis=(1, 2))
    signs = np.where(inv % 2 == 0, 1.0, -1.0).astype(np.float32)
    return P, signs


_P1, _S1 = _perms_and_signs(N1)
_P2, _S2 = _perms_and_signs(N2)

# sign-sorted perm orders: 12 even perms then 12 odd perms
_ISORT = np.concatenate([np.where(_S1 > 0)[0], np.where(_S1 < 0)[0]])
_JSORT = np.concatenate([np.where(_S2 > 0)[0], np.where(_S2 < 0)[0]])
# quadrants (a, b): pair-sign = +1 for the first two, -1 for the last two
_QUADS = [(0, 0), (1, 1), (0, 1), (1, 0)]

_cached = {}
_last_results = None  # BassKernelResults of the most recent run (for profiling)


def _build_nc(with_bias: bool):
    """Build + compile the 8-core SPMD Tile kernel (cached)."""
    key = bool(with_bias)
    if key in _cached:
        return _cached[key]

    import concourse.bacc as bacc
    import concourse.tile as tile
    from concourse import mybir

    FP = mybir.dt.float32
    TANH = mybir.ActivationFunctionType.Tanh
    AXX = mybir.AxisListType.X

    nc = bacc.Bacc(
        "TRN2",
        target_bir_lowering=False,
        debug=False,
        num_devices=NCORES,
    )

    x1f_d = nc.dram_tensor("x1f", [K1, UCOLS], FP, kind="ExternalInput").ap()
    x2f_d = nc.dram_tensor("x2f", [K2, UCOLS], FP, kind="ExternalInput").ap()
    x1w_d = nc.dram_tensor("x1w", [K1, NDENSE], FP, kind="ExternalInput").ap()
    x2w_d = nc.dram_tensor("x2w", [K2, NDENSE], FP, kind="ExternalInput").ap()
    w1_d = nc.dram_tensor("w1", [NDENSE, NDENSE], FP, kind="ExternalInput").ap()
    w2_d = nc.dram_tensor("w2", [NDENSE, NDENSE], FP, kind="ExternalInput").ap()
    if with_bias:
        b_d = nc.dram_tensor("b12", [128, 4], FP, kind="ExternalInput").ap()
    v_d = nc.dram_tensor(
        "v", [128, 2 * NGROUPS * GROUP_WALKERS * 2], FP, kind="ExternalOutput"
    ).ap()

    with tile.TileContext(nc) as tc:
        with (
            tc.tile_pool(name="consts", bufs=1) as cpool,
            tc.tile_pool(name="acts", bufs=3) as apool,
            tc.tile_pool(name="h0ring", bufs=2) as hpool,
            tc.tile_pool(name="y2ring", bufs=1) as ypool,
            tc.tile_pool(name="vout", bufs=1) as vpool,
            tc.tile_pool(name="ps", bufs=4, space="PSUM") as pspool,
        ):
            x1f = cpool.tile([K1, UCOLS], FP, tag="x1f")
            nc.sync.dma_start(x1f[:], x1f_d[:])
            x2f = cpool.tile([K2, UCOLS], FP, tag="x2f")
            nc.sync.dma_start(x2f[:], x2f_d[:])
            x1w = cpool.tile([K1, NDENSE], FP, tag="x1w")
            nc.sync.dma_start(x1w[:], x1w_d[:])
            x2w = cpool.tile([K2, NDENSE], FP, tag="x2w")
            nc.sync.dma_start(x2w[:], x2w_d[:])
            w1a = cpool.tile([128, NDENSE], FP, tag="w1a")
            nc.sync.dma_start(w1a[:], w1_d[0:128, :])
            w1b = cpool.tile([128, NDENSE], FP, tag="w1b")
            nc.sync.dma_start(w1b[:], w1_d[128:256, :])
            w2a = cpool.tile([128, NDENSE], FP, tag="w2a")
            nc.sync.dma_start(w2a[:], w2_d[0:128, :])
            w2b = cpool.tile([128, NDENSE], FP, tag="w2b")
            nc.sync.dma_start(w2b[:], w2_d[128:256, :])
            if with_bias:
                bsb = cpool.tile([128, 4], FP, tag="b12")  # b1h0 b1h1 b2h0 b2h1
                nc.sync.dma_start(bsb[:], b_d[:])

            # ---- u1s/u2s: first-layer partials, columns (walker, sorted perm)
            u1s = cpool.tile([128, 2, UCOLS], FP, tag="u1s")
            u2s = cpool.tile([128, 2, UCOLS], FP, tag="u2s")
            for (usb, xf, xw) in ((u1s, x1f, x1w), (u2s, x2f, x2w)):
                for c in range(UCOLS // TILE):
                    psu = pspool.tile([128, 2 * TILE], FP, tag="ps")
                    for h in (0, 1):
                        nc.tensor.matmul(
                            psu[:, h * TILE:(h + 1) * TILE],
                            xw[:, h * 128:(h + 1) * 128],
                            xf[:, c * TILE:(c + 1) * TILE],
                            start=True,
                            stop=True,
                        )
                    nc.vector.tensor_copy(
                        usb[:, :, c * TILE:(c + 1) * TILE],
                        psu[:].rearrange("p (h r) -> p h r", h=2),
                    )

            # v layout: [128, h, g, w_in_group, sign]
            vsb = vpool.tile([128, 2, NGROUPS, GROUP_WALKERS, 2], FP, tag="v")

            for g in range(NGROUPS):
                # ---- broadcast u1 + u2 into the 576 rows of each walker
                h0g = hpool.tile([128, 2, GROUP_ROWS], FP, tag="h0g")
                for h in (0, 1):
                    u1h = u1s[:, h, :].rearrange("p (w i) -> p w i", i=NPERM)
                    u2h = u2s[:, h, :].rearrange("p (w j) -> p w j", j=NPERM)
                    outh = h0g[:, h, :].rearrange("p (w r) -> p w r", r=NPAIR)
                    for q, (a, b2) in enumerate(_QUADS):
                        w0 = g * GROUP_WALKERS
                        w1_ = w0 + GROUP_WALKERS
                        out_ap = outh[:, :, q * QUAD:(q + 1) * QUAD].rearrange(
                            "p w (i j) -> p w i j", j=12
                        )
                        in1 = u1h[:, w0:w1_, a * 12:(a + 1) * 12].rearrange(
                            "p w (i u) -> p w i u", u=1
                        ).broadcast_to([128, GROUP_WALKERS, 12, 12])
                        in2 = u2h[:, w0:w1_, b2 * 12:(b2 + 1) * 12].rearrange(
                            "p w (u j) -> p w u j", u=1
                        ).broadcast_to([128, GROUP_WALKERS, 12, 12])
                        nc.vector.tensor_add(out_ap, in1, in2)

                y2g = ypool.tile([128, 2, GROUP_ROWS], FP, tag="y2g")
                for s in range(TILES_PER_GROUP):
                    sl = slice(s * TILE, (s + 1) * TILE)
                    h0sl = h0g[:, :, sl]  # [128, 2, 512] strided

                    # ---- layer 0 tanh, in place in the ring
                    nc.scalar.activation(h0sl, h0sl, TANH)

                    # ---- layer 1: 256 -> 256
                    ps1 = pspool.tile([128, 2 * TILE], FP, tag="ps")
                    for m in (0, 1):
                        nc.tensor.matmul(
                            ps1[:, m * TILE:(m + 1) * TILE],
                            w1a[:, m * 128:(m + 1) * 128],
                            h0g[:, 0, sl],
                            start=True,
                            stop=False,
                        )
                        nc.tensor.matmul(
                            ps1[:, m * TILE:(m + 1) * TILE],
                            w1b[:, m * 128:(m + 1) * 128],
                            h0g[:, 1, sl],
                            start=False,
                            stop=True,
                        )
                    t1 = apool.tile([128, 2 * TILE], FP, tag="t1")
                    if with_bias:
                        for m in (0, 1):
                            nc.scalar.activation(
                                t1[:, m * TILE:(m + 1) * TILE],
                                ps1[:, m * TILE:(m + 1) * TILE],
                                TANH,
                                bias=bsb[:, m:m + 1],
                            )
                    else:
                        nc.scalar.activation(t1[:], ps1[:], TANH)
                    # ---- residual 1, in place: t1 <- t1 + h0
                    nc.vector.tensor_add(
                        t1[:].rearrange("p (h r) -> p h r", h=2),
                        t1[:].rearrange("p (h r) -> p h r", h=2),
                        h0sl,
                    )

                    # ---- layer 2: 256 -> 256
                    ps2 = pspool.tile([128, 2 * TILE], FP, tag="ps")
                    for m in (0, 1):
                        nc.tensor.matmul(
                            ps2[:, m * TILE:(m + 1) * TILE],
                            w2a[:, m * 128:(m + 1) * 128],
                            t1[:, 0:TILE],
                            start=True,
                            stop=False,
                        )
                        nc.tensor.matmul(
                            ps2[:, m * TILE:(m + 1) * TILE],
                            w2b[:, m * 128:(m + 1) * 128],
                            t1[:, TILE:2 * TILE],
                            start=False,
                            stop=True,
                        )
                    t2 = apool.tile([128, 2 * TILE], FP, tag="t2")
                    if with_bias:
                        for m in (0, 1):
                            nc.scalar.activation(
                                t2[:, m * TILE:(m + 1) * TILE],
                                ps2[:, m * TILE:(m + 1) * TILE],
                                TANH,
                                bias=bsb[:, 2 + m:3 + m],
                            )
                    else:
                        nc.scalar.activation(t2[:], ps2[:], TANH)

                    # ---- residual 2: y2 = t2 + y1 into the group ring (GpSimd)
                    nc.gpsimd.tensor_add(
                        y2g[:, :, sl],
                        t2[:].rearrange("p (h r) -> p h r", h=2),
                        t1[:].rearrange("p (h r) -> p h r", h=2),
                    )

                # ---- per-walker sign-segment sums
                for h in (0, 1):
                    nc.vector.reduce_sum(
                        vsb[:, h, g, :, :],
                        y2g[:, h, :].rearrange(
                            "p (w s r) -> p w s r", s=2, r=HALFPAIR
                        ),
                        axis=AXX,
                    )

            nc.sync.dma_start(
                v_d[:],
                vsb[:].rearrange("p a b c d -> p (a b c d)"),
            )

    nc.compile()
    _cached[key] = nc
    return nc


def _build_feats(x1, x2):
    """Per-walker first-layer inputs in sign-sorted perm order.

    Returns (X1f [B, 24, 13], X2f [B, 24, 12]): X1f[b, ip] = flattened
    x1[b, P1[_ISORT[ip]]] + trailing 1.0 (carries b0); X2f likewise, no ones.
    """
    B = x1.shape[0]
    xp1 = x1[:, _P1[_ISORT], :].reshape(B, NPERM, N1 * D)
    xp2 = x2[:, _P2[_JSORT], :].reshape(B, NPERM, N2 * D)
    X1f = np.empty((B, NPERM, K1), dtype=np.float32)
    X1f[:, :, :N1 * D] = xp1
    X1f[:, :, N1 * D] = 1.0
    return X1f, np.ascontiguousarray(xp2)


def _make_in_maps(x1, x2, W0, b0, W1, b1, W2, b2):
    with_bias = bool(np.any(b1) or np.any(b2))
    X1f, X2f = _build_feats(x1, x2)
    x1w = np.ascontiguousarray(
        np.concatenate([W0[:N1 * D], b0[None, :]], axis=0)
    )  # [13, 256]
    x2w = np.ascontiguousarray(W0[N1 * D:])  # [12, 256]
    in_maps = []
    for c in range(NCORES):
        sl = slice(c * WALKERS_PER_CORE, (c + 1) * WALKERS_PER_CORE)
        m = {
            "x1f": np.ascontiguousarray(X1f[sl].reshape(UCOLS, K1).T),
            "x2f": np.ascontiguousarray(X2f[sl].reshape(UCOLS, K2).T),
            "x1w": x1w,
            "x2w": x2w,
            "w1": np.ascontiguousarray(W1),
            "w2": np.ascontiguousarray(W2),
        }
        if with_bias:
            bm = np.zeros((128, 4), dtype=np.float32)
            bm[:, 0] = b1[0:128]
            bm[:, 1] = b1[128:256]
            bm[:, 2] = b2[0:128]
            bm[:, 3] = b2[128:256]
            m["b12"] = bm
        in_maps.append(m)
    return with_bias, in_maps


def _finish(v_per_core, Wf, bf):
    """v [NCORES][128, 2*NGROUPS*GROUP_WALKERS*2] -> log|anti| [BATCH]."""
    out = np.empty((BATCH,), dtype=np.float32)
    wf64 = Wf[:, 0].astype(np.float64)
    # sum of pair signs is exactly 0, so bf drops out of the signed sum
    for c in range(NCORES):
        v = v_per_core[c].reshape(128, 2, NGROUPS, GROUP_WALKERS, 2)
        u = v[:, :, :, :, 0].astype(np.float64) - v[:, :, :, :, 1]
        u = np.transpose(u, (1, 0, 2, 3)).reshape(NDENSE, WALKERS_PER_CORE)
        anti = wf64 @ u
        out[c * WALKERS_PER_CORE:(c + 1) * WALKERS_PER_CORE] = np.log(
            np.abs(anti)
        ).astype(np.float32)
    return out


def kernel(x1, x2, W0, b0, W1, b1, W2, b2, Wf, bf):
    from concourse.bass_utils import run_bass_kernel_spmd

    x1 = np.asarray(x1, dtype=np.float32)
    x2 = np.asarray(x2, dtype=np.float32)
    W0 = np.asarray(W0, dtype=np.float32)
    b0 = np.asarray(b0, dtype=np.float32)
    W1 = np.asarray(W1, dtype=np.float32)
    b1 = np.asarray(b1, dtype=np.float32)
    W2 = np.asarray(W2, dtype=np.float32)
    b2 = np.asarray(b2, dtype=np.float32)
    Wf = np.asarray(Wf, dtype=np.float32)
    bf = np.asarray(bf, dtype=np.float32)

    with_bias, in_maps = _make_in_maps(x1, x2, W0, b0, W1, b1, W2, b2)
    nc = _build_nc(with_bias)

    res = run_bass_kernel_spmd(nc, in_maps, list(range(NCORES)))
    global _last_results
    _last_results = res

    return _finish([res.results[c]["v"] for c in range(NCORES)], Wf, bf)
